# revision 1
# baseline (speedup 1.0000x reference)
"""Trainium2 Bass kernel for nn_EnergyAE (B=64, D=12288, N=32, H=2048) on 8 cores.

Hybrid sharding:
  phase E  (model-parallel over H): encoder -> z* partial -> AllReduce z* (8KB)
  phase C  (contraction-parallel over D): Cpart = w2Ts @ w2Ts^T -> AllReduce C (16MB)
  phase S1 (data-parallel, 8 samples/core): A1 = W1*m1, P1T = C@A1T, G = P1T^T A1T,
           Prec = G/sp^2 + sig_term + I, LDLT, Lt^-1, dz, tr, logdet,
           z_s = z* + dz -> AllGather z_s (8KB)
  phase S2 (model-parallel over D): h2, x_star slice, delta, d_sq, Wd = delta@W2s^T
           -> AllReduce [Wd | d_sq] (0.5MB)
  phase S3 (data-parallel): t = W1T^T(m2*Wd), G2 = A2 C A2^T, LDLT2, fwd solve,
           d_proj_sq, recon -> out (8 per core)

Identities replacing eigvalsh/cholesky/solve_triangular (validated in model.py):
  Prec = Lt D Lt^T (unit-lower LDLT)
  sum(log eig)/2 = 0.5*sum(log D);   sum(1/eig) = ||D^-1/2 Lt^-1||_F^2
  U^-1 eps = Lt^-T (eps/sqrt(D));    t^T G2^-1 t = ||D2^-1/2 Lt2^-1 t||^2
  sig_term = (n w0 w0^T + (D-n) w1 w1^T)/2   (constant across batch)
"""
import sys

for _p in ("/opt/trn_rl_repo", "/root/.axon_site/_ro/trn_rl_repo"):
    if _p not in sys.path:
        sys.path.append(_p)

import numpy as np
from contextlib import ExitStack

import concourse.bass as bass
import concourse.mybir as mybir
import concourse.tile as tile
from concourse.masks import make_identity

B, D, N, H = 64, 12288, 32, 2048
NCORES = 8
BL = B // NCORES          # 8 local samples
HS = H // NCORES          # 256
DS = D // NCORES          # 1536
KT_H = H // 128           # 16
KT_D = D // 128           # 96
KT_DS = DS // 128         # 12
P = 128

F32 = mybir.dt.float32
F32R = mybir.dt.float32r
Alu = mybir.AluOpType
Act = mybir.ActivationFunctionType
RG = [list(range(NCORES))]


def sub_ap(t, extra_off, dims):
    """Custom free-dim AP on a [P, F] tile; dims = [[step,count],...] in elems."""
    base = t[:, 0:1]
    return bass.AP(base.tensor, base.offset + extra_off, [base.ap[0]] + dims)


def pe_T(nc, out_ps, in_ap, ident):
    """PE transpose: out_ps [f, p] = in_ap [p, f].T"""
    kp = in_ap.shape[0]
    nc.tensor.transpose(out_ps, in_ap, ident[0:kp, 0:kp])


def emit_ldlt(nc, T, OUT, invD, n=32):
    """In-place unit-lower LDLT of T [BL, n*n] (row-major per sample).
    After: strict lower of T holds unscaled columns u; diag holds D; invD = 1/D."""
    for j in range(n):
        nc.vector.reciprocal(invD[:, j:j + 1], T[:, (n + 1) * j:(n + 1) * j + 1])
        m = n - 1 - j
        if m == 0:
            break
        base = (j + 1) * n + j
        u_i = sub_ap(T, base, [[n, m], [0, m]])
        u_k = sub_ap(T, base, [[0, m], [n, m]])
        outer = sub_ap(OUT, 0, [[m, m], [1, m]])
        nc.vector.scalar_tensor_tensor(
            outer, u_i, invD[:, j:j + 1], u_k, Alu.mult, Alu.mult)
        trail = sub_ap(T, (j + 1) * (n + 1), [[n, m], [1, m]])
        nc.vector.tensor_tensor(trail, trail, outer, Alu.subtract)


def emit_ltinv(nc, LT, X, OUT, n=32):
    """X = LT^{-1} for unit-lower LT [BL, n*n]; X preset to I by caller."""
    for k in range(n - 1):
        rows = n - 1 - k
        cols = k + 1
        lcol = sub_ap(LT, (k + 1) * n + k, [[n, rows], [0, cols]])
        xrow = sub_ap(X, k * n, [[0, rows], [1, cols]])
        prod = sub_ap(OUT, 0, [[cols, rows], [1, cols]])
        nc.vector.scalar_tensor_tensor(prod, lcol, -1.0, xrow, Alu.mult, Alu.mult)
        xblk = sub_ap(X, (k + 1) * n, [[n, rows], [1, cols]])
        nc.vector.tensor_tensor(xblk, xblk, prod, Alu.add)


def emit_fwd_solve(nc, LT, y, OUT, n=32):
    """y <- LT^{-1} y for unit-lower LT [BL, n*n], y [BL, n] in place."""
    for k in range(n - 1):
        rows = n - 1 - k
        lcol = sub_ap(LT, (k + 1) * n + k, [[n, rows]])
        nc.vector.scalar_tensor_tensor(
            OUT[:, 0:rows], lcol, -1.0, y[:, k:k + 1].broadcast_to([BL, rows]),
            Alu.mult, Alu.mult)
        nc.vector.tensor_tensor(y[:, k + 1:n], y[:, k + 1:n], OUT[:, 0:rows], Alu.add)


def legalize_waits(nc, maxw=1):
    """Split multi-wait sync_info into standalone EventSemaphore instructions.

    This walrus build encodes at most one wait condition per instruction
    (codegen raises 'Too many sync wait commands' otherwise); a preceding
    same-engine wait-only instruction is semantically identical."""
    for f in nc.m.functions:
        for bb in f.blocks:
            insts = list(bb.instructions)
            out = []
            changed = False
            for inst in insts:
                si = inst.sync_info
                if si is not None and si.on_wait and len(si.on_wait) > maxw:
                    waits = list(si.on_wait)
                    # keep register-based waits on the original instruction
                    imm = [w for w in waits if w.uses_immediate]
                    reg = [w for w in waits if not w.uses_immediate]
                    keep = (reg + imm)[:maxw] if len(reg) <= maxw else reg
                    extra = [w for w in waits if w not in keep]
                    if len(keep) > maxw:
                        raise RuntimeError(f"{inst.name}: {len(keep)} register waits")
                    for w in extra:
                        ev = mybir.InstEventSemaphore(
                            name=nc.get_next_instruction_name(), ins=[], outs=[])
                        ev.engine = inst.engine
                        ev.sync_info = mybir.SyncInfo(on_wait=[w], on_update=[])
                        out.append(ev)
                    inst.sync_info = mybir.SyncInfo(
                        on_wait=keep, on_update=list(si.on_update or []))
                    changed = True
                out.append(inst)
            if changed:
                bb.instructions = out
    return nc


def build_nc():
    nc = bass.Bass()

    # ---- I/O ----
    xT = nc.dram_tensor("xT", [D, B], F32R, kind="ExternalInput")
    xmb = nc.dram_tensor("xmb", [B, DS], F32, kind="ExternalInput")
    w1es = nc.dram_tensor("w1es", [D, HS], F32R, kind="ExternalInput")
    b1es = nc.dram_tensor("b1es", [1, HS], F32, kind="ExternalInput")
    w2es = nc.dram_tensor("w2es", [HS, N], F32, kind="ExternalInput")
    b2e = nc.dram_tensor("b2e", [1, N], F32, kind="ExternalInput")
    w2Ts = nc.dram_tensor("w2Ts", [DS, H], F32R, kind="ExternalInput")
    w2s = nc.dram_tensor("w2s", [H, DS], F32R, kind="ExternalInput")
    w1 = nc.dram_tensor("w1", [N, H], F32, kind="ExternalInput")
    w1Td = nc.dram_tensor("w1Td", [H, N], F32, kind="ExternalInput")
    b1d = nc.dram_tensor("b1d", [1, H], F32, kind="ExternalInput")
    sigw = nc.dram_tensor("sigw", [1, 130], F32, kind="ExternalInput")
    sel8 = nc.dram_tensor("sel8", [B, BL], F32, kind="ExternalInput")
    epsin = nc.dram_tensor("epsin", [BL, N], F32, kind="ExternalInput")
    out = nc.dram_tensor("out", [BL, 1], F32, kind="ExternalOutput")
    dbg = nc.dram_tensor("dbg", [BL, 48], F32, kind="ExternalOutput")

    # ---- internal DRAM ----
    zst_b = nc.dram_tensor("zst_b", [B, N], F32)
    zst_sh = nc.dram_tensor("zst_sh", [B, N], F32, addr_space="Shared")
    cpart = nc.dram_tensor("cpart", [H, H], F32R)
    c_sh = nc.dram_tensor("c_sh", [H, H], F32R, addr_space="Shared")
    zs_b = nc.dram_tensor("zs_b", [BL, N], F32)
    zs_sh = nc.dram_tensor("zs_sh", [B, N], F32, addr_space="Shared")
    wd_b = nc.dram_tensor("wd_b", [B, H + 1], F32)
    wd_sh = nc.dram_tensor("wd_sh", [B, H + 1], F32, addr_space="Shared")

    with tile.TileContext(nc) as tc, ExitStack() as ctx:
        consts = ctx.enter_context(tc.tile_pool(name="consts", bufs=1))
        work = ctx.enter_context(tc.tile_pool(name="work", bufs=2))
        stream = ctx.enter_context(tc.tile_pool(name="stream", bufs=3))
        psum = ctx.enter_context(tc.tile_pool(name="psum", bufs=2, space="PSUM"))
        psum_acc = ctx.enter_context(tc.tile_pool(name="psacc", bufs=1, space="PSUM"))
        lin = ctx.enter_context(tc.tile_pool(name="lin", bufs=1))

        # ---- constants / small loads ----
        ident = consts.tile([P, P], F32)
        make_identity(nc, ident)
        ones1 = consts.tile([1, B], F32)
        nc.vector.memset(ones1, 1.0)
        sigw_sb = consts.tile([1, 130], F32)
        nc.sync.dma_start(sigw_sb, sigw[:])
        sigw_rep = consts.tile([BL, 130], F32)
        sigw_ps = psum.tile([BL, 130], F32, tag="small_ps")
        nc.tensor.matmul(sigw_ps, ones1[:, 0:BL], sigw_sb, start=True, stop=True)
        nc.vector.tensor_copy(sigw_rep, sigw_ps)
        sel8_sb = consts.tile([B, BL], F32)
        nc.sync.dma_start(sel8_sb, sel8[:])
        eps_sb = consts.tile([BL, N], F32)
        nc.sync.dma_start(eps_sb, epsin[:])
        b1es_sb = consts.tile([1, HS], F32)
        nc.sync.dma_start(b1es_sb, b1es[:])
        b2e_sb = consts.tile([1, N], F32)
        nc.sync.dma_start(b2e_sb, b2e[:])
        b1d_sb = consts.tile([1, H], F32)
        nc.sync.dma_start(b1d_sb, b1d[:])
        w1_sb = consts.tile([N, H], F32)
        nc.sync.dma_start(w1_sb, w1[:])
        w1T_sb = consts.tile([P, KT_H, N], F32)
        nc.sync.dma_start(w1T_sb, w1Td[:].rearrange("(k p) n -> p k n", p=P))

        # ================= phase C (emit first: longest pole) ==============
        pC_cm = tc.tile_pool(name="pC", bufs=1)
        pC = pC_cm.__enter__()
        w2Ts_sb = pC.tile([P, KT_DS, H], F32R, tag="w2Ts")
        w2Ts_r = w2Ts[:].rearrange("(k p) h -> p k h", p=P)
        for kt in range(KT_DS):
            nc.sync.dma_start(w2Ts_sb[:, kt, :], w2Ts_r[:, kt, :])
        for mt in range(KT_H):
            for nb in range(4):
                cps = psum.tile([P, 512], F32, tag="big_ps")
                for kt in range(KT_DS):
                    nc.tensor.matmul(
                        cps,
                        w2Ts_sb[:, kt, mt * P:(mt + 1) * P],
                        w2Ts_sb[:, kt, nb * 512:(nb + 1) * 512],
                        start=(kt == 0), stop=(kt == KT_DS - 1))
                cs = stream.tile([P, 512], F32R, tag="c_out")
                nc.scalar.copy(cs, cps)
                nc.sync.dma_start(
                    cpart[mt * P:(mt + 1) * P, nb * 512:(nb + 1) * 512], cs)
        nc.gpsimd.collective_compute("AllReduce", Alu.add, replica_groups=RG,
                                     ins=[cpart[:]], outs=[c_sh[:]])
        pC_cm.__exit__(None, None, None)
        res = ctx.enter_context(tc.tile_pool(name="res", bufs=1))

        # ================= phase E: encoder =================
        a1_ps = psum_acc.tile([B, HS], F32, tag="acc")
        w1es_r = w1es[:].rearrange("(k p) h -> p k h", p=P)
        xT_r = xT[:].rearrange("(k p) b -> p k b", p=P)
        for kt in range(KT_D):
            xt_t = stream.tile([P, B], F32R, tag="xT_t")
            nc.sync.dma_start(xt_t, xT_r[:, kt, :])
            w1t = stream.tile([P, HS], F32R, tag="w1es_t")
            nc.sync.dma_start(w1t, w1es_r[:, kt, :])
            nc.tensor.matmul(a1_ps, xt_t, w1t,
                             start=(kt == 0), stop=False)
        nc.tensor.matmul(a1_ps, ones1[:, 0:B], b1es_sb, start=False, stop=True)
        h1_sb = work.tile([B, HS], F32, tag="h1")
        nc.vector.tensor_scalar(h1_sb, a1_ps, 0.0, None, Alu.max)
        h1T_sb = work.tile([P, 2, B], F32, tag="h1T")
        for i in range(2):
            tp = psum.tile([P, B], F32, tag="small_ps")
            pe_T(nc, tp, h1_sb[:, i * P:(i + 1) * P], ident)
            nc.vector.tensor_copy(h1T_sb[:, i, :], tp)
        w2es_sb = work.tile([P, 2, N], F32, tag="w2es")
        nc.sync.dma_start(w2es_sb, w2es[:].rearrange("(k p) n -> p k n", p=P))
        zp_ps = psum.tile([B, N], F32, tag="small_ps")
        for i in range(2):
            nc.tensor.matmul(zp_ps, h1T_sb[:, i, :], w2es_sb[:, i, :],
                             start=(i == 0), stop=(i == 1))
        zp_sb = work.tile([B, N], F32, tag="zstar_part")
        nc.vector.tensor_copy(zp_sb, zp_ps)
        nc.sync.dma_start(zst_b[:], zp_sb)
        nc.gpsimd.collective_compute("AllReduce", Alu.add, replica_groups=RG,
                                     ins=[zst_b[:]], outs=[zst_sh[:]])

        # ========== z* post: transposes, local slice, sig1, masks, A1T ======
        zf_sb = work.tile([B, N], F32, tag="z_full")
        nc.sync.dma_start(zf_sb, zst_sh[:])
        zT_ps = psum.tile([N, B], F32, tag="small_ps")
        pe_T(nc, zT_ps, zf_sb, ident)
        nc.tensor.matmul(zT_ps, b2e_sb, ones1[:, 0:B], start=False, stop=True,
                         skip_group_check=True)
        zT_sb = work.tile([N, B], F32, tag="zT")        # z*^T + b2  [32, 64]
        nc.vector.tensor_copy(zT_sb, zT_ps)
        zloc_ps = psum.tile([BL, N], F32, tag="small_ps")
        nc.tensor.matmul(zloc_ps, sel8_sb, zf_sb, start=True, stop=False)
        nc.tensor.matmul(zloc_ps, ones1[:, 0:BL], b2e_sb, start=False, stop=True)
        zloc_sb = lin.tile([BL, N], F32, tag="z_loc")   # z* local + b2
        nc.vector.tensor_copy(zloc_sb, zloc_ps)

        def emit_sig(z_loc, name):
            lg = lin.tile([BL, 2, 32], F32, tag="sig_lg")
            nc.vector.tensor_tensor(
                lg, z_loc.unsqueeze(1).broadcast_to([BL, 2, 32]),
                sigw_rep[:, 0:64].rearrange("p (c n) -> p c n", c=2), Alu.mult)
            red = lin.tile([BL, 2], F32, tag=f"sig_red_{name}")
            nc.vector.tensor_reduce(red, lg, mybir.AxisListType.X, Alu.add)
            nc.vector.tensor_tensor(red, red, sigw_rep[:, 64:66], Alu.add)
            s = lin.tile([BL, 2], F32, tag=f"sig_s_{name}")
            nc.scalar.activation(s, red, Act.Exp)
            return s

        s1 = emit_sig(zloc_sb, "s1")
        invsp2 = lin.tile([BL, 1], F32, tag="invsp2")
        sp2t = lin.tile([BL, 1], F32, tag="sp2t")
        nc.vector.tensor_tensor(sp2t, s1[:, 0:1], s1[:, 0:1], Alu.mult)
        nc.vector.reciprocal(invsp2, sp2t)

        zlT_ps = psum.tile([N, BL], F32, tag="small_ps")
        pe_T(nc, zlT_ps, zloc_sb, ident)
        zlT_sb = work.tile([N, BL], F32, tag="zlT")
        nc.vector.tensor_copy(zlT_sb, zlT_ps)

        # a1T (local) -> mask m1T [P, KT_H, BL] -> A1T
        a1T_ps = psum_acc.tile([P, KT_H, BL], F32, tag="acc")
        for mt in range(KT_H):
            nc.tensor.matmul(a1T_ps[:, mt, :], w1_sb[:, mt * P:(mt + 1) * P],
                             zlT_sb, start=True, stop=False)
            nc.tensor.matmul(a1T_ps[:, mt, :], b1d_sb[:, mt * P:(mt + 1) * P],
                             ones1[:, 0:BL], start=False, stop=True)
        m1T_sb = work.tile([P, KT_H, BL], F32, tag="m1T")
        nc.vector.tensor_scalar(m1T_sb, a1T_ps, 0.0, None, Alu.is_gt)
        AT_sb = res.tile([P, KT_H, BL, N], F32R, tag="AT")
        nc.vector.tensor_tensor(
            AT_sb,
            w1T_sb.unsqueeze(2).broadcast_to([P, KT_H, BL, N]),
            m1T_sb.unsqueeze(3).broadcast_to([P, KT_H, BL, N]), Alu.mult)

        # ---- P*T = C @ A*T ; G = P*T^T A*T  (C streamed from DRAM) ----
        def emit_PG(AT, tag):
            PT_sb = res.tile([P, KT_H, BL * N], F32R, tag="PT")
            for mt in range(KT_H):
                pps = psum.tile([P, BL * N], F32, tag="mid_ps")
                for kt in range(KT_H):
                    ct = stream.tile([P, P], F32R, tag="c_t")
                    nc.sync.dma_start(
                        ct, c_sh[kt * P:(kt + 1) * P, mt * P:(mt + 1) * P])
                    nc.tensor.matmul(
                        pps, ct, AT[:, kt, :, :],
                        start=(kt == 0), stop=(kt == KT_H - 1))
                nc.scalar.copy(PT_sb[:, mt, :], pps)
            g_ps = psum_acc.tile([N, BL * N], F32, tag="acc")
            for s in range(BL):
                for kt in range(KT_H):
                    nc.tensor.matmul(
                        g_ps[:, s * N:(s + 1) * N],
                        PT_sb[:, kt, s * N:(s + 1) * N],
                        AT[:, kt, s, :],
                        start=(kt == 0), stop=(kt == KT_H - 1))
            g_sb = work.tile([N, BL * N], F32, tag="g_sb")
            nc.vector.tensor_copy(g_sb, g_ps)
            return g_sb

        g_sb = emit_PG(AT_sb, "1")
        Tm = lin.tile([BL, N * N], F32, tag="Tmat")
        for s in range(BL):
            nc.sync.dma_start(Tm[s:s + 1, :], g_sb[:, s * N:(s + 1) * N])

        # ---- Prec = G*invsp2 + sig_term + I ----
        st_ps = psum.tile([N, N], F32, tag="small_ps")
        nc.tensor.matmul(st_ps, sigw_sb[:, 66:98], sigw_sb[:, 66:98],
                         start=True, stop=False)
        nc.tensor.matmul(st_ps, sigw_sb[:, 98:130], sigw_sb[:, 98:130],
                         start=False, stop=True)
        st_sb = work.tile([N, N], F32, tag="st_sb")
        nc.vector.tensor_copy(st_sb, st_ps)
        st_flat = lin.tile([1, N * N], F32, tag="st_flat")
        nc.sync.dma_start(st_flat, st_sb)
        st_rep = lin.tile([BL, N * N], F32, tag="st_rep")
        for hh in range(2):
            st_ps2 = psum.tile([BL, 512], F32, tag="mid_ps")
            nc.tensor.matmul(st_ps2, ones1[:, 0:BL],
                             st_flat[:, hh * 512:(hh + 1) * 512], start=True, stop=True)
            nc.vector.tensor_copy(st_rep[:, hh * 512:(hh + 1) * 512], st_ps2)
        nc.vector.tensor_scalar(Tm, Tm, invsp2, None, Alu.mult)
        nc.vector.tensor_tensor(Tm, Tm, st_rep, Alu.add)
        diag1 = sub_ap(Tm, 0, [[N + 1, N]])
        nc.vector.tensor_scalar(diag1, diag1, 1.0, None, Alu.add)

        # ---- LDLT, Lt, Lt^-1, dz, tr, ld ----
        invD = lin.tile([BL, N], F32, tag="invD")
        SCR = lin.tile([BL, N * N], F32, tag="scr")
        emit_ldlt(nc, Tm, SCR, invD)
        LT = lin.tile([BL, N * N], F32, tag="LTmat")
        nc.vector.tensor_tensor(
            LT.rearrange("p (a b) -> p a b", b=N),
            Tm.rearrange("p (a b) -> p a b", b=N),
            invD.unsqueeze(1).broadcast_to([BL, N, N]), Alu.mult)
        X1 = lin.tile([BL, N * N], F32, tag="X1")
        nc.vector.memset(X1, 0.0)
        nc.vector.memset(sub_ap(X1, 0, [[N + 1, N]]), 1.0)
        emit_ltinv(nc, LT, X1, SCR)

        srD = lin.tile([BL, N], F32, tag="srD")
        nc.scalar.activation(srD, invD, Act.Sqrt)        # 1/sqrt(D)
        epss = lin.tile([BL, N], F32, tag="epss")
        nc.vector.tensor_tensor(epss, eps_sb, srD, Alu.mult)
        scrB = lin.tile([BL, N * N], F32, tag="scrB")
        nc.vector.tensor_tensor(
            scrB.rearrange("p (a b) -> p a b", b=N),
            X1.rearrange("p (a b) -> p a b", b=N),
            epss.unsqueeze(2).broadcast_to([BL, N, N]), Alu.mult)
        dz = lin.tile([BL, N], F32, tag="dz")
        nc.vector.tensor_reduce(
            dz, sub_ap(scrB, 0, [[1, N], [N, N]]), mybir.AxisListType.X, Alu.add)
        # tr = sum X1^2 * invD_row
        nc.vector.tensor_tensor(SCR, X1, X1, Alu.mult)
        trv = lin.tile([BL, 1], F32, tag="trv")
        nc.vector.tensor_tensor(
            scrB.rearrange("p (a b) -> p a b", b=N),
            SCR.rearrange("p (a b) -> p a b", b=N),
            invD.unsqueeze(2).broadcast_to([BL, N, N]), Alu.mult)
        nc.vector.tensor_reduce(trv, scrB, mybir.AxisListType.X, Alu.add)
        logs = lin.tile([BL, N], F32, tag="logs")
        ldv = lin.tile([BL, 1], F32, tag="ldv")
        nc.scalar.activation(logs, invD, Act.Ln)
        nc.vector.tensor_reduce(ldv, logs, mybir.AxisListType.X, Alu.add)  # = -sum log D
        zsq = lin.tile([BL, N], F32, tag="zsq")
        latv = lin.tile([BL, 1], F32, tag="latv")
        nc.vector.tensor_tensor(zsq, zloc_sb, zloc_sb, Alu.mult)
        nc.vector.tensor_reduce(latv, zsq, mybir.AxisListType.X, Alu.add)
        nc.vector.tensor_tensor(latv, latv, trv, Alu.add)
        nc.vector.tensor_scalar(latv, latv, 0.5, None, Alu.mult)

        zs_loc = lin.tile([BL, N], F32, tag="zs_loc")
        nc.vector.tensor_tensor(zs_loc, zloc_sb, dz, Alu.add)
        nc.sync.dma_start(zs_b[:], zs_loc)
        nc.gpsimd.collective_compute("AllGather", Alu.bypass, replica_groups=RG,
                                     ins=[zs_b[:]], outs=[zs_sh[:]])

        # ---- stage 2 prep: h2T (all), m2T (local), A2T ----
        zsf_sb = work.tile([B, N], F32, tag="z_full")
        nc.sync.dma_start(zsf_sb, zs_sh[:])
        zsT_ps = psum.tile([N, B], F32, tag="small_ps")
        pe_T(nc, zsT_ps, zsf_sb, ident)
        zsT_sb = work.tile([N, B], F32, tag="zT")
        nc.vector.tensor_copy(zsT_sb, zsT_ps)
        zslT_ps = psum.tile([N, BL], F32, tag="small_ps")
        pe_T(nc, zslT_ps, zs_loc, ident)
        zslT_sb = work.tile([N, BL], F32, tag="zlT")
        nc.vector.tensor_copy(zslT_sb, zslT_ps)

        a2T_ps = psum_acc.tile([P, KT_H, B], F32, tag="acc")
        for mt in range(KT_H):
            nc.tensor.matmul(a2T_ps[:, mt, :], w1_sb[:, mt * P:(mt + 1) * P],
                             zsT_sb, start=True, stop=False)
            nc.tensor.matmul(a2T_ps[:, mt, :], b1d_sb[:, mt * P:(mt + 1) * P],
                             ones1[:, 0:B], start=False, stop=True)
        h2T_sb = res.tile([P, KT_H, B], F32R, tag="h2T")
        nc.scalar.activation(h2T_sb, a2T_ps, Act.Relu)

        a2lT_ps = psum.tile([P, KT_H, BL], F32, tag="small_ps")
        for mt in range(KT_H):
            nc.tensor.matmul(a2lT_ps[:, mt, :], w1_sb[:, mt * P:(mt + 1) * P],
                             zslT_sb, start=True, stop=False)
            nc.tensor.matmul(a2lT_ps[:, mt, :], b1d_sb[:, mt * P:(mt + 1) * P],
                             ones1[:, 0:BL], start=False, stop=True)
        m2T_sb = work.tile([P, KT_H, BL], F32, tag="m2T")
        nc.vector.tensor_scalar(m2T_sb, a2lT_ps, 0.0, None, Alu.is_gt)
        AT2_sb = res.tile([P, KT_H, BL, N], F32R, tag="AT")   # reuse slot
        nc.vector.tensor_tensor(
            AT2_sb,
            w1T_sb.unsqueeze(2).broadcast_to([P, KT_H, BL, N]),
            m2T_sb.unsqueeze(3).broadcast_to([P, KT_H, BL, N]), Alu.mult)

        # ---- x_star slice, delta, d_sq, Wd ----
        d_sb = res.tile([B, DS], F32, tag="d_sb")
        w2s_r = w2s[:].rearrange("(k p) ds -> p k ds", p=P)
        for nb in range(3):
            xmb_t = stream.tile([B, 512], F32, tag="xmb_t")
            nc.sync.dma_start(xmb_t, xmb[:, nb * 512:(nb + 1) * 512])
            xs_ps = psum.tile([B, 512], F32, tag="big_ps")
            for kt in range(KT_H):
                wt = stream.tile([P, 512], F32R, tag="w2s_t")
                nc.sync.dma_start(wt, w2s_r[:, kt, nb * 512:(nb + 1) * 512])
                nc.tensor.matmul(xs_ps, h2T_sb[:, kt, :], wt,
                                 start=(kt == 0), stop=(kt == KT_H - 1))
            nc.vector.tensor_tensor(d_sb[:, nb * 512:(nb + 1) * 512], xmb_t,
                                    xs_ps, Alu.subtract)
        dT_sb = res.tile([P, KT_DS, B], F32R, tag="dT")
        for kt in range(KT_DS):
            tp = psum.tile([P, B], F32, tag="small_ps")
            pe_T(nc, tp, d_sb[:, kt * P:(kt + 1) * P], ident)
            nc.vector.tensor_copy(dT_sb[:, kt, :], tp)
        dsq_sb = work.tile([B, 1], F32, tag="dsq")
        dsqf = res.tile([B, DS], F32, tag="dsqf")
        nc.scalar.activation(dsqf, d_sb, Act.Square)
        nc.vector.tensor_reduce(dsq_sb, dsqf, mybir.AxisListType.X, Alu.add)
        wd_sb = res.tile([B, H + 1], F32, tag="wd")
        for mb in range(4):
            wd_ps = psum.tile([B, 512], F32, tag="big_ps")
            for kt in range(KT_DS):
                wt2 = stream.tile([P, 512], F32R, tag="w2Ts_t2")
                nc.sync.dma_start(wt2, w2Ts_r[:, kt, mb * 512:(mb + 1) * 512])
                nc.tensor.matmul(wd_ps, dT_sb[:, kt, :], wt2,
                                 start=(kt == 0), stop=(kt == KT_DS - 1))
            nc.vector.tensor_copy(wd_sb[:, mb * 512:(mb + 1) * 512], wd_ps)
        nc.vector.tensor_copy(wd_sb[:, H:H + 1], dsq_sb)
        nc.sync.dma_start(wd_b[:], wd_sb)
        nc.gpsimd.collective_compute("AllReduce", Alu.add, replica_groups=RG,
                                     ins=[wd_b[:]], outs=[wd_sh[:]])

        # ---- local Wd/dsq; t = w1T^T @ (m2 . Wd) ----
        wdf_sb = res.tile([B, H + 1], F32, tag="wd")   # reuse slot after AR
        nc.sync.dma_start(wdf_sb, wd_sh[:])
        wdl_sb = res.tile([BL, H + 1], F32, tag="wd_loc")
        for c in range(4):
            wp = psum.tile([BL, 512], F32, tag="mid_ps")
            nc.tensor.matmul(wp, sel8_sb, wdf_sb[:, c * 512:(c + 1) * 512],
                             start=True, stop=True)
            nc.vector.tensor_copy(wdl_sb[:, c * 512:(c + 1) * 512], wp)
        wp1 = psum.tile([BL, 1], F32, tag="small_ps")
        nc.tensor.matmul(wp1, sel8_sb, wdf_sb[:, H:H + 1], start=True, stop=True)
        dsql = lin.tile([BL, 1], F32, tag="dsql")
        nc.vector.tensor_copy(dsql, wp1)
        wdlT_sb = work.tile([P, KT_H, BL], F32, tag="wdlT")
        for kt in range(KT_H):
            tp2 = psum.tile([P, BL], F32, tag="small_ps")
            pe_T(nc, tp2, wdl_sb[:, kt * P:(kt + 1) * P], ident)
            nc.vector.tensor_copy(wdlT_sb[:, kt, :], tp2)
        mwdT_sb = work.tile([P, KT_H, BL], F32, tag="mwdT")
        nc.vector.tensor_tensor(mwdT_sb, wdlT_sb, m2T_sb, Alu.mult)
        t_ps = psum.tile([N, BL], F32, tag="small_ps")
        for kt in range(KT_H):
            nc.tensor.matmul(t_ps, w1T_sb[:, kt, :], mwdT_sb[:, kt, :],
                             start=(kt == 0), stop=(kt == KT_H - 1))
        t_sb = work.tile([N, BL], F32, tag="t_sb")
        nc.vector.tensor_copy(t_sb, t_ps)
        tl_ps = psum.tile([BL, N], F32, tag="small_ps")
        pe_T(nc, tl_ps, t_sb, ident)
        y = lin.tile([BL, N], F32, tag="y")
        nc.vector.tensor_copy(y, tl_ps)

        # ---- G2, LDLT2, solve ----
        g2_sb = emit_PG(AT2_sb, "2")
        Tm2 = lin.tile([BL, N * N], F32, tag="Tmat")   # reuse slot
        for s in range(BL):
            nc.sync.dma_start(Tm2[s:s + 1, :], g2_sb[:, s * N:(s + 1) * N])
        invD2 = lin.tile([BL, N], F32, tag="invD2")
        emit_ldlt(nc, Tm2, SCR, invD2)
        LT2 = lin.tile([BL, N * N], F32, tag="LTmat")  # reuse slot
        nc.vector.tensor_tensor(
            LT2.rearrange("p (a b) -> p a b", b=N),
            Tm2.rearrange("p (a b) -> p a b", b=N),
            invD2.unsqueeze(1).broadcast_to([BL, N, N]), Alu.mult)
        emit_fwd_solve(nc, LT2, y, SCR)
        ysq = lin.tile([BL, N], F32, tag="ysq")
        yw = lin.tile([BL, N], F32, tag="yw")
        dproj = lin.tile([BL, 1], F32, tag="dproj")
        nc.vector.tensor_tensor(ysq, y, y, Alu.mult)
        nc.vector.tensor_tensor(yw, ysq, invD2, Alu.mult)
        nc.vector.tensor_reduce(dproj, yw, mybir.AxisListType.X, Alu.add)

        # ---- recon / output ----
        s2 = emit_sig(zs_loc, "s2")
        sq2 = lin.tile([BL, 2], F32, tag="sq2")
        nc.vector.tensor_tensor(sq2, s2, s2, Alu.mult)
        nc.vector.tensor_scalar(sq2, sq2, 2.0, None, Alu.mult)
        inv2 = lin.tile([BL, 2], F32, tag="inv2")
        nc.vector.reciprocal(inv2, sq2)     # [1/(2sp2^2), 1/(2sv2^2)]
        logs2 = lin.tile([BL, 2], F32, tag="logs2")
        logw = lin.tile([BL, 2], F32, tag="logw")
        nc.vector.memset(logw[:, 0:1], float(N))
        nc.vector.memset(logw[:, 1:2], float(D - N))
        nc.scalar.activation(logs2, s2, Act.Ln)
        logterm = lin.tile([BL, 1], F32, tag="logterm")
        junk2 = lin.tile([BL, 2], F32, tag="junk2")
        nc.vector.tensor_tensor(junk2, logs2, logw, Alu.mult)
        nc.vector.tensor_reduce(logterm, junk2, mybir.AxisListType.X, Alu.add)
        isub = lin.tile([BL, 1], F32, tag="isub")
        nc.vector.tensor_tensor(isub, inv2[:, 0:1], inv2[:, 1:2], Alu.subtract)
        recon = lin.tile([BL, 1], F32, tag="recon")
        nc.vector.tensor_tensor(recon, dproj, isub, Alu.mult)
        p2t = lin.tile([BL, 1], F32, tag="p2t")
        nc.vector.tensor_tensor(p2t, dsql, inv2[:, 1:2], Alu.mult)
        nc.vector.tensor_tensor(recon, recon, p2t, Alu.add)
        nc.vector.tensor_tensor(recon, recon, logterm, Alu.add)
        nc.vector.tensor_scalar(ldv, ldv, -0.5, None, Alu.mult)
        ov = lin.tile([BL, 1], F32, tag="ov")
        nc.vector.tensor_tensor(ov, recon, latv, Alu.add)
        nc.vector.tensor_tensor(ov, ov, ldv, Alu.add)
        nc.vector.tensor_scalar(ov, ov, 1.0 / D, None, Alu.mult)
        nc.sync.dma_start(out[:], ov)

        # ---- debug ----
        dbg_sb = lin.tile([BL, 48], F32, tag="dbg")
        nc.vector.tensor_copy(dbg_sb[:, 0:N], zs_loc)
        nc.vector.tensor_copy(dbg_sb[:, 32:33], ldv)
        nc.vector.tensor_copy(dbg_sb[:, 33:34], trv)
        nc.vector.tensor_copy(dbg_sb[:, 34:35], dproj)
        nc.vector.tensor_copy(dbg_sb[:, 35:36], dsql)
        nc.vector.tensor_copy(dbg_sb[:, 36:37], recon)
        nc.vector.tensor_copy(dbg_sb[:, 37:38], latv)
        nc.vector.tensor_copy(dbg_sb[:, 38:40], s2)
        nc.vector.tensor_copy(dbg_sb[:, 40:42], s1)
        nc.sync.dma_start(dbg[:], dbg_sb)

    legalize_waits(nc)
    return nc


def shard_inputs(inputs):
    """Host-side prep: returns in_maps list for the 8 cores."""
    x = np.ascontiguousarray(np.asarray(inputs["x"], np.float32))
    eps = np.ascontiguousarray(np.asarray(inputs["eps"], np.float32))
    eW1 = np.ascontiguousarray(np.asarray(inputs["enc_W1"], np.float32))
    eb1 = np.asarray(inputs["enc_b1"], np.float32)
    eW2 = np.ascontiguousarray(np.asarray(inputs["enc_W2"], np.float32))
    eb2 = np.asarray(inputs["enc_b2"], np.float32)
    dW1 = np.ascontiguousarray(np.asarray(inputs["dec_W1"], np.float32))
    db1 = np.asarray(inputs["dec_b1"], np.float32)
    dW2 = np.ascontiguousarray(np.asarray(inputs["dec_W2"], np.float32))
    db2 = np.asarray(inputs["dec_b2"], np.float32)
    sW = np.asarray(inputs["sig_W"], np.float32)
    sb = np.asarray(inputs["sig_b"], np.float32)

    xT = np.ascontiguousarray(x.T)
    dW2T = np.ascontiguousarray(dW2.T)
    dW1T = np.ascontiguousarray(dW1.T)
    sigv = np.zeros((1, 130), np.float32)
    sigv[0, 0:32] = sW[:, 0]
    sigv[0, 32:64] = sW[:, 1]
    sigv[0, 64:66] = sb
    sigv[0, 66:98] = sW[:, 0] * np.sqrt(N / 2.0)
    sigv[0, 98:130] = sW[:, 1] * np.sqrt((D - N) / 2.0)

    maps = []
    for k in range(NCORES):
        sel = np.zeros((B, BL), np.float32)
        for i in range(BL):
            sel[k * BL + i, i] = 1.0
        maps.append({
            "xT": xT,
            "xmb": np.ascontiguousarray(
                x[:, k * DS:(k + 1) * DS] - db2[None, k * DS:(k + 1) * DS]),
            "w1es": np.ascontiguousarray(eW1[:, k * HS:(k + 1) * HS]),
            "b1es": np.ascontiguousarray(eb1[None, k * HS:(k + 1) * HS]),
            "w2es": np.ascontiguousarray(eW2[k * HS:(k + 1) * HS, :]),
            "b2e": np.ascontiguousarray(eb2[None, :]),
            "w2Ts": np.ascontiguousarray(dW2T[k * DS:(k + 1) * DS, :]),
            "w2s": np.ascontiguousarray(dW2[:, k * DS:(k + 1) * DS]),
            "w1": dW1,
            "w1Td": dW1T,
            "b1d": np.ascontiguousarray(db1[None, :]),
            "sigw": sigv,
            "sel8": sel,
            "epsin": np.ascontiguousarray(eps[k * BL:(k + 1) * BL, :]),
        })
    return maps


_NC_CACHE = None


def kernel(**inputs) -> np.ndarray:
    global _NC_CACHE
    from concourse.bass_utils import run_bass_kernel_spmd
    if _NC_CACHE is None:
        _NC_CACHE = build_nc()
    nc = _NC_CACHE
    maps = shard_inputs(inputs)
    res = run_bass_kernel_spmd(nc, maps, list(range(NCORES)))
    outs = [res.results[k]["out"].reshape(BL) for k in range(NCORES)]
    return np.concatenate(outs).astype(np.float32)



# revision 35
# speedup vs baseline: 2.3097x; 2.3097x over previous
"""Trainium2 Bass kernel for nn_EnergyAE (B=64, D=12288, N=32, H=2048) on 8 cores.

Hybrid sharding, bf16 matmuls (fp32 vector math):
  phase E  (model-parallel over H): encoder -> z* partial (bf16, folded into
           C-chunk0 AllReduce below)
  phase C  (contraction-parallel over D): upper-triangular 512-col blocks of
           Cpart = w2Ts @ w2Ts^T, AllReduced in 4 bf16 chunks pipelined with
           the build; mirrored to full C in SBUF locally after readback.
           C stays SBUF-resident for both PG stages.
  phase S1 (data-parallel, 8 samples/core): A1 = W1*m1, P1T = C@A1T,
           G = P1T^T A1T (batched 4 samples/matmul), Prec, LDLT, Lt^-1, dz,
           tr, logdet, z_s = z* + dz -> AllGather z_s (8KB)
  phase S2 (model-parallel over D): h2, x_star slice, delta, d_sq,
           Wd = delta@W2s^T (w2Ts reused from SBUF) -> AllReduce [Wd | d_sq]
  phase S3 (data-parallel): t = W1T^T(m2*Wd), G2 = A2 C A2^T, LDLT2,
           fwd solve, d_proj_sq, recon -> out (8 per core)

Identities replacing eigvalsh/cholesky/solve_triangular:
  Prec = Lt D Lt^T (unit-lower LDLT)
  sum(log eig)/2 = 0.5*sum(log D);   sum(1/eig) = ||D^-1/2 Lt^-1||_F^2
  U^-1 eps = Lt^-T (eps/sqrt(D));    t^T G2^-1 t = ||D2^-1/2 Lt2^-1 t||^2
  sig_term = (n w0 w0^T + (D-n) w1 w1^T)/2   (constant across batch)
"""
import sys

for _p in ("/opt/trn_rl_repo", "/root/.axon_site/_ro/trn_rl_repo"):
    if _p not in sys.path:
        sys.path.append(_p)

import numpy as np
import ml_dtypes
from contextlib import ExitStack

import concourse.bass as bass
import concourse.mybir as mybir
import concourse.tile as tile
from concourse.masks import make_identity

B, D, N, H = 64, 12288, 32, 2048
NCORES = 8
BL = B // NCORES          # 8 local samples
HS = H // NCORES          # 256
DS = D // NCORES          # 1536
KT_H = H // 128           # 16
KT_D = D // 128           # 96
KT_DS = DS // 128         # 12
P = 128
# upper-triangular C slab layout: col-block q holds block-rows 0..4q+3
CUP_OFF = [0, 4, 12, 24]          # slab index offset per col-block
CUP_NROW = [4, 8, 12, 16]         # slabs per col-block
CUP_TOT = 40
ZROWS = 4                         # z* partial occupies cup rows 0..3 (4x512)

F32 = mybir.dt.float32
BF16 = mybir.dt.bfloat16
Alu = mybir.AluOpType
Act = mybir.ActivationFunctionType
RG = [list(range(NCORES))]


def sub_ap(t, extra_off, dims):
    """Custom free-dim AP on a [P, F] tile; dims = [[step,count],...] in elems."""
    base = t[:, 0:1]
    return bass.AP(base.tensor, base.offset + extra_off, [base.ap[0]] + dims)


def pe_T(nc, out_ps, in_ap, ident):
    """PE transpose: out_ps [f, p] = in_ap [p, f].T"""
    kp = in_ap.shape[0]
    nc.tensor.transpose(out_ps, in_ap, ident[0:kp, 0:kp])


def emit_ldlt(nc, T, OUT, invD, n=32):
    """In-place unit-lower LDLT of T [BL, n*n] (row-major per sample).
    After: strict lower of T holds unscaled columns u; diag holds D; invD = 1/D."""
    for j in range(n):
        nc.vector.reciprocal(invD[:, j:j + 1], T[:, (n + 1) * j:(n + 1) * j + 1])
        m = n - 1 - j
        if m == 0:
            break
        base = (j + 1) * n + j
        u_i = sub_ap(T, base, [[n, m], [0, m]])
        u_k = sub_ap(T, base, [[0, m], [n, m]])
        outer = sub_ap(OUT, 0, [[m, m], [1, m]])
        nc.vector.scalar_tensor_tensor(
            outer, u_i, invD[:, j:j + 1], u_k, Alu.mult, Alu.mult)
        trail = sub_ap(T, (j + 1) * (n + 1), [[n, m], [1, m]])
        nc.vector.tensor_tensor(trail, trail, outer, Alu.subtract)


def emit_ltinv(nc, LT, X, OUT, n=32):
    """X = LT^{-1} for unit-lower LT [BL, n*n]; X preset to I by caller."""
    for k in range(n - 1):
        rows = n - 1 - k
        cols = k + 1
        lcol = sub_ap(LT, (k + 1) * n + k, [[n, rows], [0, cols]])
        xrow = sub_ap(X, k * n, [[0, rows], [1, cols]])
        prod = sub_ap(OUT, 0, [[cols, rows], [1, cols]])
        nc.vector.scalar_tensor_tensor(prod, lcol, -1.0, xrow, Alu.mult, Alu.mult)
        xblk = sub_ap(X, (k + 1) * n, [[n, rows], [1, cols]])
        nc.vector.tensor_tensor(xblk, xblk, prod, Alu.add)


def emit_fwd_solve(nc, LT, y, OUT, n=32):
    """y <- LT^{-1} y for unit-lower LT [BL, n*n], y [BL, n] in place."""
    for k in range(n - 1):
        rows = n - 1 - k
        lcol = sub_ap(LT, (k + 1) * n + k, [[n, rows]])
        nc.vector.scalar_tensor_tensor(
            OUT[:, 0:rows], lcol, -1.0, y[:, k:k + 1].broadcast_to([BL, rows]),
            Alu.mult, Alu.mult)
        nc.vector.tensor_tensor(y[:, k + 1:n], y[:, k + 1:n], OUT[:, 0:rows], Alu.add)


def legalize_waits(nc, maxw=1):
    """Split multi-wait sync_info into standalone EventSemaphore instructions."""
    for f in nc.m.functions:
        for bb in f.blocks:
            insts = list(bb.instructions)
            out = []
            changed = False
            for inst in insts:
                si = inst.sync_info
                if si is not None and si.on_wait and len(si.on_wait) > maxw:
                    waits = list(si.on_wait)
                    imm = [w for w in waits if w.uses_immediate]
                    reg = [w for w in waits if not w.uses_immediate]
                    keep = (reg + imm)[:maxw] if len(reg) <= maxw else reg
                    extra = [w for w in waits if w not in keep]
                    if len(keep) > maxw:
                        raise RuntimeError(f"{inst.name}: {len(keep)} register waits")
                    for w in extra:
                        ev = mybir.InstEventSemaphore(
                            name=nc.get_next_instruction_name(), ins=[], outs=[])
                        ev.engine = inst.engine
                        ev.sync_info = mybir.SyncInfo(on_wait=[w], on_update=[])
                        out.append(ev)
                    inst.sync_info = mybir.SyncInfo(
                        on_wait=keep, on_update=list(si.on_update or []))
                    changed = True
                out.append(inst)
            if changed:
                bb.instructions = out
    return nc


def build_nc():
    nc = bass.Bass()

    # ---- I/O ----
    xT = nc.dram_tensor("xT", [D, B], BF16, kind="ExternalInput")
    xmb = nc.dram_tensor("xmb", [B, DS], BF16, kind="ExternalInput")
    w1es = nc.dram_tensor("w1es", [D, HS], BF16, kind="ExternalInput")
    b1es = nc.dram_tensor("b1es", [1, HS], BF16, kind="ExternalInput")
    w2es = nc.dram_tensor("w2es", [HS, N], BF16, kind="ExternalInput")
    b2e = nc.dram_tensor("b2e", [1, N], BF16, kind="ExternalInput")
    w2Ts = nc.dram_tensor("w2Ts", [DS, H], BF16, kind="ExternalInput")
    w2s = nc.dram_tensor("w2s", [H, DS], BF16, kind="ExternalInput")
    w1 = nc.dram_tensor("w1", [N, H], BF16, kind="ExternalInput")
    w1Td = nc.dram_tensor("w1Td", [H, N], BF16, kind="ExternalInput")
    b1d = nc.dram_tensor("b1d", [1, H], BF16, kind="ExternalInput")
    sigw = nc.dram_tensor("sigw", [1, 130], F32, kind="ExternalInput")
    sel8 = nc.dram_tensor("sel8", [B, BL], BF16, kind="ExternalInput")
    epsin = nc.dram_tensor("epsin", [BL, N], F32, kind="ExternalInput")
    out = nc.dram_tensor("out", [BL, 1], F32, kind="ExternalOutput")

    # ---- internal DRAM ----
    # chunk q: upper-C col-block q slabs (+ z* partial rows in chunk 0)
    cup = [nc.dram_tensor(f"cup{q}", [(ZROWS if q == 0 else 0) + CUP_NROW[q] * P, 512],
                          BF16) for q in range(4)]
    cup_sh = [nc.dram_tensor(f"cup_sh{q}", [(ZROWS if q == 0 else 0) + CUP_NROW[q] * P, 512],
                             BF16, addr_space="Shared") for q in range(4)]
    zs_b = nc.dram_tensor("zs_b", [BL, N], F32)
    zs_sh = nc.dram_tensor("zs_sh", [B, N], F32, addr_space="Shared")
    wd_b = nc.dram_tensor("wd_b", [B, H + 1], F32)
    wds_b = nc.dram_tensor("wds_b", [BL, H + 1], F32)

    with tile.TileContext(nc) as tc, ExitStack() as ctx:
        consts = ctx.enter_context(tc.tile_pool(name="consts", bufs=1))
        work = ctx.enter_context(tc.tile_pool(name="work", bufs=2))
        stream = ctx.enter_context(tc.tile_pool(name="stream", bufs=3))
        psum = ctx.enter_context(tc.tile_pool(name="psum", bufs=2, space="PSUM"))
        psum_acc = ctx.enter_context(tc.tile_pool(name="psacc", bufs=1, space="PSUM"))
        lin = ctx.enter_context(tc.tile_pool(name="lin", bufs=1))
        res = ctx.enter_context(tc.tile_pool(name="res", bufs=1))

        # ---- constants / small loads ----
        identb = consts.tile([P, P], BF16)
        make_identity(nc, identb)
        ones1 = consts.tile([1, B], F32)
        nc.vector.memset(ones1, 1.0)
        onesb = consts.tile([1, B], BF16)
        nc.vector.memset(onesb, 1.0)
        sigw_sb = consts.tile([1, 130], F32)
        nc.sync.dma_start(sigw_sb, sigw[:])
        sigw_rep = consts.tile([BL, 130], F32)
        sigw_ps = psum.tile([BL, 130], F32, tag="small_ps")
        nc.tensor.matmul(sigw_ps, ones1[:, 0:BL], sigw_sb, start=True, stop=True)
        nc.vector.tensor_copy(sigw_rep, sigw_ps)
        sel8_sb = consts.tile([B, BL], BF16)
        nc.sync.dma_start(sel8_sb, sel8[:])
        eps_sb = consts.tile([BL, N], F32)
        nc.sync.dma_start(eps_sb, epsin[:])
        b1es_sb = consts.tile([1, HS], BF16)
        nc.sync.dma_start(b1es_sb, b1es[:])
        b2e_sb = consts.tile([1, N], BF16)
        nc.sync.dma_start(b2e_sb, b2e[:])
        # decoder bias as per-partition columns [P, KT_H] (+ negated copy)
        b1dcol = consts.tile([P, KT_H], BF16)
        nc.sync.dma_start(b1dcol, b1d[:].rearrange("o (k p) -> p (o k)", p=P))
        nb1col = consts.tile([P, KT_H], F32)
        nc.vector.tensor_scalar(nb1col, b1dcol, -1.0, None, Alu.mult)
        w1_sb = consts.tile([N, H], BF16)
        nc.sync.dma_start(w1_sb, w1[:])
        w1T_sb = consts.tile([P, KT_H, N], BF16)
        nc.sync.dma_start(w1T_sb, w1Td[:].rearrange("(k p) n -> p k n", p=P))

        # ---- resident weights: w2Ts (used by phase C and Wd) ----
        w2Ts_sb = res.tile([P, KT_DS, H], BF16, tag="w2Ts")
        w2Ts_r = w2Ts[:].rearrange("(k p) h -> p k h", p=P)
        for kt in range(KT_DS):
            nc.sync.dma_start(w2Ts_sb[:, kt, :], w2Ts_r[:, kt, :])

        # ================= phase E: encoder (z* partial -> cup rows 0:4) ====
        a1_ps = psum_acc.tile([B, HS], F32, tag="acc")
        w1es_r = w1es[:].rearrange("(k p) h -> p k h", p=P)
        xT_r = xT[:].rearrange("(k p) b -> p k b", p=P)
        for kt in range(KT_D):
            xt_t = stream.tile([P, B], BF16, tag="xT_t")
            nc.sync.dma_start(xt_t, xT_r[:, kt, :])
            w1t = stream.tile([P, HS], BF16, tag="w1es_t")
            nc.sync.dma_start(w1t, w1es_r[:, kt, :])
            nc.tensor.matmul(a1_ps, xt_t, w1t, start=(kt == 0), stop=False)
        nc.tensor.matmul(a1_ps, onesb[:, 0:B], b1es_sb, start=False, stop=True)
        h1_sb = work.tile([B, HS], BF16, tag="h1")
        nc.vector.tensor_scalar(h1_sb, a1_ps, 0.0, None, Alu.max)
        h1T_sb = work.tile([P, 2, B], BF16, tag="h1T")
        for i in range(2):
            tp = psum.tile([P, B], BF16, tag="t_ps")
            pe_T(nc, tp, h1_sb[:, i * P:(i + 1) * P], identb)
            nc.vector.tensor_copy(h1T_sb[:, i, :], tp)
        w2es_sb = work.tile([P, 2, N], BF16, tag="w2es")
        nc.sync.dma_start(w2es_sb, w2es[:].rearrange("(k p) n -> p k n", p=P))
        zp_ps = psum.tile([B, N], F32, tag="small_ps")
        for i in range(2):
            nc.tensor.matmul(zp_ps, h1T_sb[:, i, :], w2es_sb[:, i, :],
                             start=(i == 0), stop=(i == 1))
        zp_sb = work.tile([B, N], BF16, tag="zstar_part")
        nc.vector.tensor_copy(zp_sb, zp_ps)
        nc.sync.dma_start(cup[0][0:ZROWS, :], zp_sb)

        # ========= phase C: upper C slabs + chunked AllReduce ==============
        # col-block q: cols [512q, 512q+512), block-rows j in 0..4q+3
        for q in range(4):
            zr = ZROWS if q == 0 else 0
            for j in range(4 * q + 4):
                cps = psum.tile([P, 512], F32, tag="big_ps")
                for kd in range(KT_DS):
                    nc.tensor.matmul(
                        cps,
                        w2Ts_sb[:, kd, j * P:(j + 1) * P],
                        w2Ts_sb[:, kd, 512 * q:512 * (q + 1)],
                        start=(kd == 0), stop=(kd == KT_DS - 1))
                cs = work.tile([P, 512], BF16, tag="c_out")
                nc.scalar.copy(cs, cps)
                r0 = zr + j * P
                nc.sync.dma_start(cup[q][r0:r0 + P, :], cs)
            # AllReduce this chunk (chunk 0 also carries the z* partial)
            nc.gpsimd.collective_compute(
                "AllReduce", Alu.add, replica_groups=RG,
                ins=[cup[q][:]], outs=[cup_sh[q][:]])

        # ---- z* full readback (available after chunk-0 AR) ----
        zf_sb = work.tile([B, N], BF16, tag="z_full")
        nc.sync.dma_start(zf_sb, cup_sh[0][0:ZROWS, :])

        # ---- z* post: local slice, sig1, masks, A1T (overlaps C build) ----
        zlT_ps = psum.tile([N, BL], F32, tag="small_ps")
        nc.tensor.matmul(zlT_ps, zf_sb, sel8_sb, start=True, stop=False)
        nc.tensor.matmul(zlT_ps, b2e_sb, onesb[:, 0:BL], start=False, stop=True)
        zlT_sb = work.tile([N, BL], BF16, tag="zlT")   # (z*loc + b2)^T
        nc.vector.tensor_copy(zlT_sb, zlT_ps)
        zloc_ps = psum.tile([BL, N], F32, tag="small_ps")
        nc.tensor.matmul(zloc_ps, sel8_sb, zf_sb, start=True, stop=False)
        nc.tensor.matmul(zloc_ps, onesb[:, 0:BL], b2e_sb, start=False, stop=True)
        zloc_sb = lin.tile([BL, N], F32, tag="z_loc")   # z* local + b2
        nc.vector.tensor_copy(zloc_sb, zloc_ps)

        def emit_sig(z_loc, name):
            lg = lin.tile([BL, 2, 32], F32, tag="sig_lg")
            nc.vector.tensor_tensor(
                lg, z_loc.unsqueeze(1).broadcast_to([BL, 2, 32]),
                sigw_rep[:, 0:64].rearrange("p (c n) -> p c n", c=2), Alu.mult)
            red = lin.tile([BL, 2], F32, tag=f"sig_red_{name}")
            nc.vector.tensor_reduce(red, lg, mybir.AxisListType.X, Alu.add)
            nc.vector.tensor_tensor(red, red, sigw_rep[:, 64:66], Alu.add)
            s = lin.tile([BL, 2], F32, tag=f"sig_s_{name}")
            nc.scalar.activation(s, red, Act.Exp)
            return s

        s1 = emit_sig(zloc_sb, "s1")
        invsp2 = lin.tile([BL, 1], F32, tag="invsp2")
        sp2t = lin.tile([BL, 1], F32, tag="sp2t")
        nc.vector.tensor_tensor(sp2t, s1[:, 0:1], s1[:, 0:1], Alu.mult)
        nc.vector.reciprocal(invsp2, sp2t)

        # a1T (local) -> mask m1T [P, KT_H, BL] -> A1T  (mask: a1 > -b1)
        m1T_sb = work.tile([P, KT_H, BL], BF16, tag="m1T")
        for mt in range(KT_H):
            aps = psum.tile([P, BL], F32, tag="small_ps")
            nc.tensor.matmul(aps, w1_sb[:, mt * P:(mt + 1) * P],
                             zlT_sb, start=True, stop=True)
            nc.vector.tensor_tensor(
                m1T_sb[:, mt, :], aps,
                nb1col[:, mt:mt + 1].broadcast_to([P, BL]), Alu.is_gt)
        AT_sb = res.tile([P, KT_H, BL, N], BF16, tag="AT")
        nc.vector.tensor_tensor(
            AT_sb,
            w1T_sb.unsqueeze(2).broadcast_to([P, KT_H, BL, N]),
            m1T_sb.unsqueeze(3).broadcast_to([P, KT_H, BL, N]), Alu.mult)

        # ---- C readback into SBUF + local mirror of lower blocks ----
        csb = res.tile([P, KT_H, H], BF16, tag="csb")
        for q in range(4):
            zr = ZROWS if q == 0 else 0
            shr = cup_sh[q][zr:, :].rearrange("(s p) c -> p s c", p=P)
            # upper col-block q -> csb[:, 0:4q+4, 512q:512q+512]
            nc.sync.dma_start(
                csb[:, 0:4 * q + 4, 512 * q:512 * (q + 1)], shr)
        # mirror: block (i, j) with i//4 > j//4  <-  transpose of (j, i)
        for qi in range(1, 4):
            for i in range(4 * qi, 4 * qi + 4):
                for j in range(4 * qi):
                    tp = psum.tile([P, P], BF16, tag="t_ps")
                    pe_T(nc, tp, csb[:, j, i * P:(i + 1) * P], identb)
                    nc.scalar.copy(csb[:, i, j * P:(j + 1) * P], tp)

        # ---- P*T = C @ A*T ; G = P*T^T A*T  (C resident in SBUF) ----
        def emit_PG(AT, tag):
            PT_sb = res.tile([P, KT_H, BL * N], BF16, tag="PT")
            for mt in range(KT_H):
                pps = psum.tile([P, BL * N], F32, tag="big_ps")
                for kt in range(KT_H):
                    nc.tensor.matmul(
                        pps, csb[:, kt, mt * P:(mt + 1) * P], AT[:, kt, :, :],
                        start=(kt == 0), stop=(kt == KT_H - 1))
                nc.scalar.copy(PT_sb[:, mt, :], pps)
            g_sb = work.tile([P, 2, P], F32, tag="g_sb")
            for grp in range(2):
                g_ps = psum.tile([P, P], F32, tag="big_ps")
                for kt in range(KT_H):
                    nc.tensor.matmul(
                        g_ps,
                        PT_sb[:, kt, grp * P:(grp + 1) * P],
                        AT[:, kt, 4 * grp:4 * grp + 4, :],
                        start=(kt == 0), stop=(kt == KT_H - 1))
                nc.vector.tensor_copy(g_sb[:, grp, :], g_ps)
            return g_sb

        # ---- Prec = G*invsp2 + sig_term + I ----
        # preset Tm with replicated sig_term while PG runs
        st_ps = psum.tile([N, N], F32, tag="small_ps")
        nc.tensor.matmul(st_ps, sigw_sb[:, 66:98], sigw_sb[:, 66:98],
                         start=True, stop=False)
        nc.tensor.matmul(st_ps, sigw_sb[:, 98:130], sigw_sb[:, 98:130],
                         start=False, stop=True)
        st_sb = work.tile([N, N], F32, tag="st_sb")
        nc.vector.tensor_copy(st_sb, st_ps)
        st_flat = lin.tile([1, N * N], F32, tag="st_flat")
        nc.sync.dma_start(st_flat, st_sb)
        Tm = lin.tile([BL, N * N], F32, tag="Tmat")
        for hh in range(2):
            st_ps2 = psum.tile([BL, 512], F32, tag="big_ps")
            nc.tensor.matmul(st_ps2, ones1[:, 0:BL],
                             st_flat[:, hh * 512:(hh + 1) * 512], start=True, stop=True)
            nc.vector.tensor_copy(Tm[:, hh * 512:(hh + 1) * 512], st_ps2)
        # add diag I up front
        diag1 = sub_ap(Tm, 0, [[N + 1, N]])
        nc.vector.tensor_scalar(diag1, diag1, 1.0, None, Alu.add)

        g_sb = emit_PG(AT_sb, "1")
        SCR = lin.tile([BL, N * N], F32, tag="scr")
        for s in range(BL):
            grp, sl = s // 4, s % 4
            nc.sync.dma_start(
                SCR[s:s + 1, :],
                g_sb[sl * N:(sl + 1) * N, grp, sl * N:(sl + 1) * N])
        nc.vector.scalar_tensor_tensor(Tm, SCR, invsp2, Tm, Alu.mult, Alu.add)

        # ---- LDLT, Lt, Lt^-1, dz, tr, ld ----
        invD = lin.tile([BL, N], F32, tag="invD")
        emit_ldlt(nc, Tm, SCR, invD)
        LT = lin.tile([BL, N * N], F32, tag="LTmat")
        nc.vector.tensor_tensor(
            LT.rearrange("p (a b) -> p a b", b=N),
            Tm.rearrange("p (a b) -> p a b", b=N),
            invD.unsqueeze(1).broadcast_to([BL, N, N]), Alu.mult)
        X1 = lin.tile([BL, N * N], F32, tag="X1")
        nc.vector.memset(X1, 0.0)
        nc.vector.memset(sub_ap(X1, 0, [[N + 1, N]]), 1.0)
        emit_ltinv(nc, LT, X1, SCR)

        srD = lin.tile([BL, N], F32, tag="srD")
        nc.scalar.activation(srD, invD, Act.Sqrt)        # 1/sqrt(D)
        epss = lin.tile([BL, N], F32, tag="epss")
        nc.vector.tensor_tensor(epss, eps_sb, srD, Alu.mult)
        scrB = lin.tile([BL, N * N], F32, tag="LTmat")  # LT dead after ltinv
        nc.vector.tensor_tensor(
            scrB.rearrange("p (a b) -> p a b", b=N),
            X1.rearrange("p (a b) -> p a b", b=N),
            epss.unsqueeze(2).broadcast_to([BL, N, N]), Alu.mult)
        dz = lin.tile([BL, N], F32, tag="dz")
        nc.vector.tensor_reduce(
            dz, sub_ap(scrB, 0, [[1, N], [N, N]]), mybir.AxisListType.X, Alu.add)
        # tr = sum X1^2 * invD_row
        nc.vector.tensor_tensor(SCR, X1, X1, Alu.mult)
        trv = lin.tile([BL, 1], F32, tag="trv")
        nc.vector.tensor_tensor(
            scrB.rearrange("p (a b) -> p a b", b=N),
            SCR.rearrange("p (a b) -> p a b", b=N),
            invD.unsqueeze(2).broadcast_to([BL, N, N]), Alu.mult)
        nc.vector.tensor_reduce(trv, scrB, mybir.AxisListType.X, Alu.add)
        logs = lin.tile([BL, N], F32, tag="logs")
        ldv = lin.tile([BL, 1], F32, tag="ldv")
        nc.scalar.activation(logs, invD, Act.Ln)
        nc.vector.tensor_reduce(ldv, logs, mybir.AxisListType.X, Alu.add)  # -sum log D
        zsq = lin.tile([BL, N], F32, tag="zsq")
        latv = lin.tile([BL, 1], F32, tag="latv")
        nc.vector.tensor_tensor(zsq, zloc_sb, zloc_sb, Alu.mult)
        nc.vector.tensor_reduce(latv, zsq, mybir.AxisListType.X, Alu.add)
        nc.vector.tensor_tensor(latv, latv, trv, Alu.add)
        nc.vector.tensor_scalar(latv, latv, 0.5, None, Alu.mult)

        zs_loc = lin.tile([BL, N], F32, tag="zs_loc")
        nc.vector.tensor_tensor(zs_loc, zloc_sb, dz, Alu.add)
        nc.sync.dma_start(zs_b[:], zs_loc)
        nc.gpsimd.collective_compute("AllGather", Alu.bypass, replica_groups=RG,
                                     ins=[zs_b[:]], outs=[zs_sh[:]])

        # ---- stage 2 prep: h2T (all), m2T (local), A2T ----
        zsf_sb = work.tile([B, N], F32, tag="z_full2")
        nc.sync.dma_start(zsf_sb, zs_sh[:])
        zsf_bf = work.tile([B, N], BF16, tag="z_full2b")
        nc.vector.tensor_copy(zsf_bf, zsf_sb)
        zs_bf = lin.tile([BL, N], BF16, tag="zs_locb")
        nc.vector.tensor_copy(zs_bf, zs_loc)
        zsT_ps = psum.tile([N, B], BF16, tag="t_ps")
        pe_T(nc, zsT_ps, zsf_bf, identb)
        zsT_sb = work.tile([N, B], BF16, tag="zT2")
        nc.vector.tensor_copy(zsT_sb, zsT_ps)
        zslT_ps = psum.tile([N, BL], BF16, tag="t_ps")
        pe_T(nc, zslT_ps, zs_bf, identb)
        zslT_sb = work.tile([N, BL], BF16, tag="zlT2")
        nc.vector.tensor_copy(zslT_sb, zslT_ps)

        h2T_sb = res.tile([P, KT_H, B], BF16, tag="h2T")
        for mt in range(KT_H):
            aps = psum.tile([P, B], F32, tag="small_ps")
            nc.tensor.matmul(aps, w1_sb[:, mt * P:(mt + 1) * P],
                             zsT_sb, start=True, stop=True)
            nc.scalar.activation(h2T_sb[:, mt, :], aps, Act.Relu,
                                 bias=b1dcol[:, mt:mt + 1])

        m2T_sb = work.tile([P, KT_H, BL], BF16, tag="m2T")
        for mt in range(KT_H):
            aps = psum.tile([P, BL], F32, tag="small_ps")
            nc.tensor.matmul(aps, w1_sb[:, mt * P:(mt + 1) * P],
                             zslT_sb, start=True, stop=True)
            nc.vector.tensor_tensor(
                m2T_sb[:, mt, :], aps,
                nb1col[:, mt:mt + 1].broadcast_to([P, BL]), Alu.is_gt)
        AT2_sb = res.tile([P, KT_H, BL, N], BF16, tag="AT")   # reuse slot
        nc.vector.tensor_tensor(
            AT2_sb,
            w1T_sb.unsqueeze(2).broadcast_to([P, KT_H, BL, N]),
            m2T_sb.unsqueeze(3).broadcast_to([P, KT_H, BL, N]), Alu.mult)

        # ---- x_star slice, delta, d_sq, Wd (w2Ts from SBUF) ----
        d_sb = res.tile([B, DS], BF16, tag="d_sb")
        w2s_r = w2s[:].rearrange("(k p) ds -> p k ds", p=P)
        for nb in range(3):
            xmb_t = stream.tile([B, 512], BF16, tag="xmb_t")
            nc.sync.dma_start(xmb_t, xmb[:, nb * 512:(nb + 1) * 512])
            xs_ps = psum.tile([B, 512], F32, tag="big_ps")
            for kt in range(KT_H):
                wt = stream.tile([P, 512], BF16, tag="w2s_t")
                nc.sync.dma_start(wt, w2s_r[:, kt, nb * 512:(nb + 1) * 512])
                nc.tensor.matmul(xs_ps, h2T_sb[:, kt, :], wt,
                                 start=(kt == 0), stop=(kt == KT_H - 1))
            nc.vector.tensor_tensor(d_sb[:, nb * 512:(nb + 1) * 512], xmb_t,
                                    xs_ps, Alu.subtract)
        dT_sb = res.tile([P, KT_DS, B], BF16, tag="dT")
        for kt in range(KT_DS):
            tp = psum.tile([P, B], BF16, tag="t_ps")
            pe_T(nc, tp, d_sb[:, kt * P:(kt + 1) * P], identb)
            nc.vector.tensor_copy(dT_sb[:, kt, :], tp)
        dsq_sb = work.tile([B, 1], F32, tag="dsq")
        # d_sq = rowsum(delta^2); squares written in place (d_sb dead after dT)
        nc.scalar.activation(d_sb, d_sb, Act.Square, accum_out=dsq_sb)
        wd_sb = res.tile([B, H + 1], F32, tag="wd")
        for mb in range(4):
            wd_ps = psum.tile([B, 512], F32, tag="big_ps")
            for kt in range(KT_DS):
                nc.tensor.matmul(wd_ps, dT_sb[:, kt, :],
                                 w2Ts_sb[:, kt, mb * 512:(mb + 1) * 512],
                                 start=(kt == 0), stop=(kt == KT_DS - 1))
            nc.vector.tensor_copy(wd_sb[:, mb * 512:(mb + 1) * 512], wd_ps)
        nc.vector.tensor_copy(wd_sb[:, H:H + 1], dsq_sb)
        nc.sync.dma_start(wd_b[:], wd_sb)
        nc.gpsimd.collective_compute("ReduceScatter", Alu.add, replica_groups=RG,
                                     ins=[wd_b[:]], outs=[wds_b[:]])

        # ---- G2 on PE while Wd AllReduce runs ----
        g2_sb = emit_PG(AT2_sb, "2")
        Tm2 = lin.tile([BL, N * N], F32, tag="Tmat")   # reuse slot
        for s in range(BL):
            grp, sl = s // 4, s % 4
            nc.sync.dma_start(
                Tm2[s:s + 1, :],
                g2_sb[sl * N:(sl + 1) * N, grp, sl * N:(sl + 1) * N])

        # ---- local Wd/dsq arrive directly via ReduceScatter ----
        wdl_sb = res.tile([BL, H + 1], F32, tag="wd_loc")
        nc.sync.dma_start(wdl_sb, wds_b[:])
        dsql = lin.tile([BL, 1], F32, tag="dsql")
        nc.vector.tensor_copy(dsql, wdl_sb[:, H:H + 1])
        wdl_bf = res.tile([BL, H], BF16, tag="wd_locb")
        nc.vector.tensor_copy(wdl_bf, wdl_sb[:, 0:H])
        wdlT_sb = work.tile([P, KT_H, BL], BF16, tag="wdlT")
        for kt in range(KT_H):
            tp2 = psum.tile([P, BL], BF16, tag="t_ps")
            pe_T(nc, tp2, wdl_bf[:, kt * P:(kt + 1) * P], identb)
            nc.vector.tensor_copy(wdlT_sb[:, kt, :], tp2)
        mwdT_sb = work.tile([P, KT_H, BL], BF16, tag="mwdT")
        nc.vector.tensor_tensor(mwdT_sb, wdlT_sb, m2T_sb, Alu.mult)
        # y[bl, n] = sum_h mwdT[h, bl] * w1T[h, n]  (t, already transposed)
        y_ps = psum.tile([BL, N], F32, tag="small_ps")
        for kt in range(KT_H):
            nc.tensor.matmul(y_ps, mwdT_sb[:, kt, :], w1T_sb[:, kt, :],
                             start=(kt == 0), stop=(kt == KT_H - 1))
        y = lin.tile([BL, N], F32, tag="y")
        nc.vector.tensor_copy(y, y_ps)

        # ---- LDLT2, solve ----
        invD2 = lin.tile([BL, N], F32, tag="invD2")
        emit_ldlt(nc, Tm2, SCR, invD2)
        LT2 = lin.tile([BL, N * N], F32, tag="LTmat")  # reuse slot
        nc.vector.tensor_tensor(
            LT2.rearrange("p (a b) -> p a b", b=N),
            Tm2.rearrange("p (a b) -> p a b", b=N),
            invD2.unsqueeze(1).broadcast_to([BL, N, N]), Alu.mult)
        emit_fwd_solve(nc, LT2, y, SCR)
        ysq = lin.tile([BL, N], F32, tag="ysq")
        yw = lin.tile([BL, N], F32, tag="yw")
        dproj = lin.tile([BL, 1], F32, tag="dproj")
        nc.vector.tensor_tensor(ysq, y, y, Alu.mult)
        nc.vector.tensor_tensor(yw, ysq, invD2, Alu.mult)
        nc.vector.tensor_reduce(dproj, yw, mybir.AxisListType.X, Alu.add)

        # ---- recon / output ----
        s2 = emit_sig(zs_loc, "s2")
        sq2 = lin.tile([BL, 2], F32, tag="sq2")
        nc.vector.tensor_tensor(sq2, s2, s2, Alu.mult)
        nc.vector.tensor_scalar(sq2, sq2, 2.0, None, Alu.mult)
        inv2 = lin.tile([BL, 2], F32, tag="inv2")
        nc.vector.reciprocal(inv2, sq2)     # [1/(2sp2^2), 1/(2sv2^2)]
        logs2 = lin.tile([BL, 2], F32, tag="logs2")
        logw = lin.tile([BL, 2], F32, tag="logw")
        nc.vector.memset(logw[:, 0:1], float(N))
        nc.vector.memset(logw[:, 1:2], float(D - N))
        nc.scalar.activation(logs2, s2, Act.Ln)
        logterm = lin.tile([BL, 1], F32, tag="logterm")
        junk2 = lin.tile([BL, 2], F32, tag="junk2")
        nc.vector.tensor_tensor(junk2, logs2, logw, Alu.mult)
        nc.vector.tensor_reduce(logterm, junk2, mybir.AxisListType.X, Alu.add)
        isub = lin.tile([BL, 1], F32, tag="isub")
        nc.vector.tensor_tensor(isub, inv2[:, 0:1], inv2[:, 1:2], Alu.subtract)
        recon = lin.tile([BL, 1], F32, tag="recon")
        nc.vector.tensor_tensor(recon, dproj, isub, Alu.mult)
        p2t = lin.tile([BL, 1], F32, tag="p2t")
        nc.vector.tensor_tensor(p2t, dsql, inv2[:, 1:2], Alu.mult)
        nc.vector.tensor_tensor(recon, recon, p2t, Alu.add)
        nc.vector.tensor_tensor(recon, recon, logterm, Alu.add)
        nc.vector.tensor_scalar(ldv, ldv, -0.5, None, Alu.mult)
        ov = lin.tile([BL, 1], F32, tag="ov")
        nc.vector.tensor_tensor(ov, recon, latv, Alu.add)
        nc.vector.tensor_tensor(ov, ov, ldv, Alu.add)
        nc.vector.tensor_scalar(ov, ov, 1.0 / D, None, Alu.mult)
        nc.sync.dma_start(out[:], ov)

    legalize_waits(nc)
    return nc


def shard_inputs(inputs):
    """Host-side prep: returns in_maps list for the 8 cores."""
    bf = ml_dtypes.bfloat16
    x = np.ascontiguousarray(np.asarray(inputs["x"], np.float32))
    eps = np.ascontiguousarray(np.asarray(inputs["eps"], np.float32))
    eW1 = np.ascontiguousarray(np.asarray(inputs["enc_W1"], np.float32))
    eb1 = np.asarray(inputs["enc_b1"], np.float32)
    eW2 = np.ascontiguousarray(np.asarray(inputs["enc_W2"], np.float32))
    eb2 = np.asarray(inputs["enc_b2"], np.float32)
    dW1 = np.ascontiguousarray(np.asarray(inputs["dec_W1"], np.float32))
    db1 = np.asarray(inputs["dec_b1"], np.float32)
    dW2 = np.ascontiguousarray(np.asarray(inputs["dec_W2"], np.float32))
    db2 = np.asarray(inputs["dec_b2"], np.float32)
    sW = np.asarray(inputs["sig_W"], np.float32)
    sb = np.asarray(inputs["sig_b"], np.float32)

    xT = np.ascontiguousarray(x.T).astype(bf)
    dW2T = np.ascontiguousarray(dW2.T)
    dW1T = np.ascontiguousarray(dW1.T).astype(bf)
    dW1b = dW1.astype(bf)
    sigv = np.zeros((1, 130), np.float32)
    sigv[0, 0:32] = sW[:, 0]
    sigv[0, 32:64] = sW[:, 1]
    sigv[0, 64:66] = sb
    sigv[0, 66:98] = sW[:, 0] * np.sqrt(N / 2.0)
    sigv[0, 98:130] = sW[:, 1] * np.sqrt((D - N) / 2.0)

    maps = []
    for k in range(NCORES):
        sel = np.zeros((B, BL), np.float32)
        for i in range(BL):
            sel[k * BL + i, i] = 1.0
        maps.append({
            "xT": xT,
            "xmb": np.ascontiguousarray(
                x[:, k * DS:(k + 1) * DS]
                - db2[None, k * DS:(k + 1) * DS]).astype(bf),
            "w1es": np.ascontiguousarray(eW1[:, k * HS:(k + 1) * HS]).astype(bf),
            "b1es": np.ascontiguousarray(eb1[None, k * HS:(k + 1) * HS]).astype(bf),
            "w2es": np.ascontiguousarray(eW2[k * HS:(k + 1) * HS, :]).astype(bf),
            "b2e": np.ascontiguousarray(eb2[None, :]).astype(bf),
            "w2Ts": np.ascontiguousarray(dW2T[k * DS:(k + 1) * DS, :]).astype(bf),
            "w2s": np.ascontiguousarray(dW2[:, k * DS:(k + 1) * DS]).astype(bf),
            "w1": dW1b,
            "w1Td": dW1T,
            "b1d": np.ascontiguousarray(db1[None, :]).astype(bf),
            "sigw": sigv,
            "sel8": sel.astype(bf),
            "epsin": np.ascontiguousarray(eps[k * BL:(k + 1) * BL, :]),
        })
    return maps


_NC_CACHE = None


def kernel(**inputs) -> np.ndarray:
    global _NC_CACHE
    from concourse.bass_utils import run_bass_kernel_spmd
    if _NC_CACHE is None:
        _NC_CACHE = build_nc()
    nc = _NC_CACHE
    maps = shard_inputs(inputs)
    res = run_bass_kernel_spmd(nc, maps, list(range(NCORES)))
    outs = [res.results[k]["out"].reshape(BL) for k in range(NCORES)]
    return np.concatenate(outs).astype(np.float32)
